# revision 26
# baseline (speedup 1.0000x reference)
"""Trainium2 Bass kernel for nn_Attention (B=2,T=8,N=512,C=768,H=12).

Data-parallel: 16 (b,t) slices -> 2 per core x 8 cores. Scheme (b2):
  - x uploaded PRE-TRANSPOSED from host in fp8 (xT8) + scaled fp8
    residual (xT8r): no on-device transposes/copies for x.
  - qk-gen: fp8 DoubleRow 1-pass (256-contraction, 0.5 cy/col):
    q psum = x8 @ (Wq*scale*QSq), k psum = x8 @ (Wk*QSk); drained to
    plain BF16 q/k tiles (exact logits up to the fp8 matmul noise).
  - scores: plain bf16 matmuls (64-contraction): S = kb^T qb; costs the
    same PE as the DR alternative but halves the psum drains and error.
  - v-gen: fp8 DR 3-pass (x8@Wv*VS + x8@Wv*VSres + x8r@Wv*VS/XRS) ->
    v*VS bf16; l-column = VS so softmax normalization cancels the scale.
  - pv/outT/proj: bf16.
PE ~81600 cy/slice (vs 104640 baseline). ACT = exp only (+ a few
copies), drains batched wide on DVE, mask-mults split DVE/Pool.
"""
import sys

sys.path.insert(0, "/opt/trn_rl_repo")

import numpy as np
import ml_dtypes
import concourse.bacc as bacc
import concourse.mybir as mybir
import concourse.tile as tile
from concourse.bass import AP, broadcast_tensor_aps
from concourse.bass_utils import run_bass_kernel_spmd
from concourse.masks import make_identity

B, T, N, C = 2, 8, 512, 768
H = 12
Dh = C // H            # 64
SL = 2                 # slices per core
NCORES = 8
NC4 = N // 128         # 4 n-chunks
CC6 = C // 128         # 6 c-chunks
CP3 = CC6 // 2         # 3 c-chunk pairs (DoubleRow planes)
F32 = mybir.dt.float32
BF16 = mybir.dt.bfloat16
FP8 = mybir.dt.float8e4
EXP = mybir.ActivationFunctionType.Exp
DR = mybir.MatmulPerfMode.DoubleRow
NP8 = ml_dtypes.float8_e4m3
NPBF = ml_dtypes.bfloat16

QSq = 512.0            # q psum scale (incl. Dh^-0.5 fold)
QSk = 64.0             # k psum scale
XRS = 8.0              # x residual upscale
VS = 512.0             # v psum scale (cancelled via l-column = VS)
EXPSCALE = 1.0 / (QSq * QSk)

_cache = {}


def build_nc():
    nc = bacc.Bacc()
    xt8 = nc.dram_tensor("xt8", [SL, 128, CC6 * N], FP8, kind="ExternalInput")
    xt8r = nc.dram_tensor("xt8r", [SL, 128, CC6 * N], FP8, kind="ExternalInput")
    wqk8 = nc.dram_tensor("wqk8", [CP3, 128, 2 * 2 * C], FP8, kind="ExternalInput")
    wv8 = nc.dram_tensor("wv8", [CP3, 128, 2 * C], FP8, kind="ExternalInput")
    wv8r = nc.dram_tensor("wv8r", [CP3, 128, 2 * C], FP8, kind="ExternalInput")
    wv8b = nc.dram_tensor("wv8b", [CP3, 128, 2 * C], FP8, kind="ExternalInput")
    wp = nc.dram_tensor("wp", [C, C], BF16, kind="ExternalInput")
    emw = nc.dram_tensor("emw", [128, NC4 * N], BF16, kind="ExternalInput")
    y = nc.dram_tensor("y", [SL, N, C], F32, kind="ExternalOutput")

    with tile.TileContext(nc) as tc:
        with (
            tc.tile_pool(name="wpool", bufs=1) as wpool,
            tc.tile_pool(name="sb", bufs=1) as sb,
            tc.tile_pool(name="ps", bufs=1, space="PSUM") as ps,
        ):
            # ---- persistent weights / inputs ----
            qkw = [wpool.tile([128, 2 * 2 * C], FP8, tag=f"qkw{p}", name=f"qkw{p}")
                   for p in range(CP3)]
            vw = [wpool.tile([128, 2 * C], FP8, tag=f"vw{p}", name=f"vw{p}")
                  for p in range(CP3)]
            vwr = [wpool.tile([128, 2 * C], FP8, tag=f"vwr{p}", name=f"vwr{p}")
                   for p in range(CP3)]
            vwb = [wpool.tile([128, 2 * C], FP8, tag=f"vwb{p}", name=f"vwb{p}")
                   for p in range(CP3)]
            projw = [wpool.tile([128, C], BF16, tag=f"projw{cc}", name=f"projw{cc}")
                     for cc in range(CC6)]
            emwt = wpool.tile([128, NC4 * N], BF16, tag="emw", name="emw")
            xts = [wpool.tile([128, CC6 * N], FP8, tag=f"xt{s}", name=f"xt{s}")
                   for s in range(SL)]
            xtrs = [wpool.tile([128, CC6 * N], FP8, tag=f"xtr{s}", name=f"xtr{s}")
                    for s in range(SL)]

            identf = wpool.tile([128, 128], F32, tag="identf", name="identf")
            make_identity(nc, identf[:])
            ident = wpool.tile([128, 128], BF16, tag="ident", name="ident")
            with nc.allow_low_precision(reason="bf16 identity"):
                nc.vector.tensor_copy(ident[:], identf[:])
            # warm the exp table early so the first scores-exp is fast
            warm = wpool.tile([128, 1], F32, tag="warm", name="warm")
            nc.scalar.activation(warm[:], identf[:, 0:1], EXP)

            def emit_weight_dmas():
                # critical first: qkw + xt0 spread across 4 engine sequencers
                nc.sync.dma_start(qkw[0][:], wqk8[0, :, :])
                nc.scalar.dma_start(qkw[1][:], wqk8[1, :, :])
                nc.gpsimd.dma_start(qkw[2][:], wqk8[2, :, :])
                hw = CC6 * N // 2
                nc.gpsimd.dma_start(xts[0][:, 0:hw], xt8[0, :, 0:hw])
                nc.gpsimd.dma_start(xts[0][:, hw:], xt8[0, :, hw:])
                nc.gpsimd.dma_start(emwt[:], emw[:, :])
                # v weights + x residual next (needed a few us in)
                for p in range(CP3):
                    nc.sync.dma_start(vw[p][:], wv8[p, :, :])
                    nc.sync.dma_start(vwr[p][:], wv8r[p, :, :])
                    nc.sync.dma_start(vwb[p][:], wv8b[p, :, :])
                nc.gpsimd.dma_start(xtrs[0][:, 0:hw], xt8r[0, :, 0:hw])
                nc.gpsimd.dma_start(xtrs[0][:, hw:], xt8r[0, :, hw:])

            def emit_projw_dmas():
                for cc in range(CC6):
                    nc.gpsimd.dma_start(projw[cc][:], wp[128 * cc:128 * (cc + 1), :])

            def emit_x_dmas(s):
                hw = CC6 * N // 2
                nc.sync.dma_start(xts[s][:, 0:hw], xt8[s, :, 0:hw])
                nc.sync.dma_start(xts[s][:, hw:], xt8[s, :, hw:])
                nc.sync.dma_start(xtrs[s][:, 0:hw], xt8r[s, :, 0:hw])
                nc.sync.dma_start(xtrs[s][:, hw:], xt8r[s, :, hw:])

            # ---- per-slice state ----
            cnt = {"msk": 0, "kb": 0}
            qbs = [[None] * CC6 for _ in range(SL)]
            kbs = [[None] * CC6 for _ in range(SL)]
            v3s = [[None] * NC4 for _ in range(SL)]
            outs = [[None] * NC4 for _ in range(SL)]
            outTs = [None] * SL
            ptiles = {}
            pots = [None, None]

            def get(lst, i, mk):
                if lst[i] is None:
                    lst[i] = mk()
                return lst[i]

            def xview(s, resid=False):
                t = (xtrs if resid else xts)[s]
                return t[:].rearrange("p (g n) -> p g n", g=CC6)

            def emit_qkgen(s, j):
                """q,k chunk j (heads 2j,2j+1): 6 DR matmuls + 2 bf16 drains."""
                pq = ps.tile([128, 1024], F32, tag="big", name=f"pqk{s}_{j}", bufs=3)
                xv = xview(s)
                for i in range(CP3):
                    ccp = (j + i) % CP3
                    rhs = xv[:, 2 * ccp:2 * ccp + 2, :]
                    wv_ = qkw[ccp][:].rearrange("p (u d) -> p u d", u=2)
                    nc.tensor.matmul(pq[:, 0:512], wv_[:, :, 128 * j:128 * (j + 1)],
                                     rhs, start=(i == 0), stop=(i == CP3 - 1),
                                     perf_mode=DR)
                    nc.tensor.matmul(pq[:, 512:1024],
                                     wv_[:, :, C + 128 * j:C + 128 * (j + 1)],
                                     rhs, start=(i == 0), stop=(i == CP3 - 1),
                                     perf_mode=DR)
                qb = get(qbs[s], j, lambda: sb.tile(
                    [128, N], BF16, tag="qb", name=f"qb_{s}_{j}", bufs=7))
                kb = get(kbs[s], j, lambda: sb.tile(
                    [128, N], BF16, tag="kb", name=f"kb_{s}_{j}", bufs=7))
                cnt["kb"] += 1
                keng = nc.scalar.copy if cnt["kb"] % 2 else nc.vector.tensor_copy
                with nc.allow_low_precision(reason="bf16 q/k"):
                    nc.vector.tensor_copy(qb[:], pq[:, 0:512])
                    keng(kb[:], pq[:, 512:1024])

            def emit_v(s, mc):
                """v*VS for token chunk mc: 18 DR matmuls (3 passes) + 1 drain."""
                pv = ps.tile([128, 1024], F32, tag="big", name=f"pv{s}_{mc}", bufs=3)
                va, vb = pv[:, 0:512], pv[:, 512:768]
                xv, xvr = xview(s), xview(s, resid=True)
                steps = []
                for w_ in (vw, vwr):
                    for i in range(CP3):
                        steps.append((xv, w_[(mc + i) % CP3], (mc + i) % CP3))
                for i in range(CP3):
                    steps.append((xvr, vwb[(mc + i) % CP3], (mc + i) % CP3))
                for i, (xsrc, w, ccp) in enumerate(steps):
                    lhsT = xsrc[:, 2 * ccp:2 * ccp + 2, 128 * mc:128 * (mc + 1)]
                    wv_ = w[:].rearrange("p (u d) -> p u d", u=2)
                    nc.tensor.matmul(va, lhsT, wv_[:, :, 0:512],
                                     start=(i == 0), stop=(i == len(steps) - 1),
                                     perf_mode=DR)
                    nc.tensor.matmul(vb, lhsT, wv_[:, :, 512:768],
                                     start=(i == 0), stop=(i == len(steps) - 1),
                                     perf_mode=DR)
                v3 = get(v3s[s], mc, lambda: sb.tile(
                    [128, H * (Dh + 1)], BF16, tag="v3", name=f"v3_{s}_{mc}", bufs=8))
                v3r = v3[:].rearrange("p (h e) -> p h e", e=Dh + 1)
                with nc.allow_low_precision(reason="bf16 v"):
                    nc.vector.tensor_copy(
                        v3r[:, :, 0:Dh],
                        pv[:, 0:C].rearrange("p (h e) -> p h e", e=Dh))
                nc.gpsimd.memset(v3r[:, :, Dh:Dh + 1], VS)

            def emit_scores(s, h):
                """S^T for head h (bf16) -> exp -> P[128, 2048] (col 512*mc+n)."""
                j = h // 2
                hb = 64 * (h % 2)
                qb = qbs[s][j][hb:hb + 64, :]
                kb = kbs[s][j]
                ptile = sb.tile([128, NC4 * N], BF16, tag="ptile",
                                name=f"pt{s}_{h}", bufs=7)
                for half in range(2):
                    pst = ps.tile([128, 1024], F32, tag="big",
                                  name=f"ps{s}_{h}_{half}", bufs=3)
                    for m2 in range(2):
                        mc = 2 * half + m2
                        nc.tensor.matmul(pst[:, 512 * m2:512 * (m2 + 1)],
                                         kb[hb:hb + 64, 128 * mc:128 * (mc + 1)],
                                         qb, start=True, stop=True)
                    nc.scalar.activation(ptile[:, 1024 * half:1024 * (half + 1)],
                                         pst[:], EXP, scale=EXPSCALE)
                # mask-mult split: DVE and Pool each take half (parallel)
                with nc.allow_low_precision(reason="bf16 P"):
                    nc.vector.tensor_mul(ptile[:, 0:1024], ptile[:, 0:1024],
                                         emwt[:, 0:1024])
                    nc.gpsimd.tensor_mul(ptile[:, 1024:2048], ptile[:, 1024:2048],
                                         emwt[:, 1024:2048])
                ptiles[(s, h)] = ptile

            def emit_pv(s, h):
                v3 = v3s[s]
                g = h // 3
                hg = h % 3
                ptile = ptiles.pop((s, h))
                if hg == 0:
                    for p in range(2):
                        pots[p] = ps.tile([128, 2 * 3 * (Dh + 1)], F32, tag="pot",
                                          name=f"pot{s}_{g}_{p}", bufs=2)
                for n4 in range(NC4):
                    pot = pots[n4 // 2][:, 195 * (n4 % 2):195 * (n4 % 2) + 195]
                    for mc in range(NC4):
                        nc.tensor.matmul(pot[:, 65 * hg:65 * (hg + 1)],
                                         ptile[:, 512 * mc + 128 * n4:
                                               512 * mc + 128 * (n4 + 1)],
                                         v3[mc][:, 65 * h:65 * (h + 1)],
                                         start=(mc == 0), stop=(mc == NC4 - 1),
                                         skip_group_check=True)
                if hg == 2:
                    for n4 in range(NC4):
                        pot = pots[n4 // 2][:, 195 * (n4 % 2):195 * (n4 % 2) + 195]
                        potv = pot.rearrange("p (h e) -> p h e", e=Dh + 1)
                        lr = sb.tile([128, 3], F32, tag="lr",
                                     name=f"lr{s}_{g}_{n4}", bufs=4)
                        nc.vector.reciprocal(
                            lr[:].rearrange("p (h e) -> p h e", e=1),
                            potv[:, :, Dh:Dh + 1])
                        out = get(outs[s], n4, lambda n4=n4: sb.tile(
                            [128, C], BF16, tag="out", name=f"out_{s}_{n4}", bufs=8))
                        b0, b1 = broadcast_tensor_aps(
                            potv[:, :, 0:Dh],
                            lr[:].rearrange("p (h e) -> p h e", e=1))
                        with nc.allow_low_precision(reason="bf16 out"):
                            nc.vector.tensor_mul(
                                out[:, 192 * g:192 * (g + 1)].rearrange(
                                    "p (h e) -> p h e", e=Dh), b0, b1)

            def emit_outT(s, n4, ccs):
                out = outs[s][n4]
                outT = get(outTs, s, lambda: sb.tile(
                    [128, CC6 * N], BF16, tag="outT", name=f"outT_{s}", bufs=2))
                pt = ps.tile([128, 1024], BF16, tag="big",
                             name=f"ot{s}_{n4}_{min(ccs)}", bufs=3)
                for i, cc in enumerate(ccs):
                    nc.tensor.transpose(pt[:, 128 * i:128 * (i + 1)],
                                        out[:, 128 * cc:128 * (cc + 1)], ident[:])
                oTv = outT[:].rearrange("p (cc n) -> p cc n", cc=CC6)
                dst = oTv[:, min(ccs):min(ccs) + len(ccs),
                          128 * n4:128 * (n4 + 1)]
                src = pt[:, 0:128 * len(ccs)].rearrange(
                    "p (cc n) -> p cc n", n=128)
                with nc.allow_low_precision(reason="bf16 outT"):
                    nc.vector.tensor_copy(dst, src)

            def emit_proj(s, n4):
                outT = outTs[s]
                oTv = outT[:].rearrange("p (cc n) -> p cc n", cc=CC6)
                osb = sb.tile([128, C], F32, tag="osb", name=f"osb{s}_{n4}", bufs=3)
                if s == 1 and n4 == NC4 - 1:
                    # final unit: 3 narrow psum groups so the drain pipelines
                    bounds = [(0, 320), (320, 640), (640, 768)]
                    for third, (c0, c1) in enumerate(bounds):
                        w = c1 - c0
                        pr = ps.tile([128, 1024], F32, tag="big",
                                     name=f"pr{s}_{n4}_{third}", bufs=3)
                        for cc in range(CC6):
                            lhsT = oTv[:, cc, 128 * n4:128 * (n4 + 1)]
                            nc.tensor.matmul(pr[:, 0:w], lhsT, projw[cc][:, c0:c1],
                                             start=(cc == 0), stop=(cc == CC6 - 1))
                        ceng = (nc.vector.tensor_copy, nc.scalar.copy,
                                nc.vector.tensor_copy)[third]
                        ceng(osb[:, c0:c1], pr[:, 0:w])
                        deng = (nc.sync, nc.gpsimd, nc.scalar)[third]
                        deng.dma_start(y[s, 128 * n4:128 * (n4 + 1), c0:c1],
                                       osb[:, c0:c1])
                    return
                pr = ps.tile([128, 1024], F32, tag="big",
                             name=f"pr{s}_{n4}", bufs=3)
                pra, prb = pr[:, 0:512], pr[:, 512:768]
                for cc in range(CC6):
                    lhsT = oTv[:, cc, 128 * n4:128 * (n4 + 1)]
                    nc.tensor.matmul(pra, lhsT, projw[cc][:, 0:512],
                                     start=(cc == 0), stop=(cc == CC6 - 1))
                    nc.tensor.matmul(prb, lhsT, projw[cc][:, 512:768],
                                     start=(cc == 0), stop=(cc == CC6 - 1))
                nc.vector.tensor_copy(osb[:, 0:512], pra)
                nc.sync.dma_start(y[s, 128 * n4:128 * (n4 + 1), 0:512],
                                  osb[:, 0:512])
                nc.scalar.copy(osb[:, 512:768], prb)
                nc.gpsimd.dma_start(y[s, 128 * n4:128 * (n4 + 1), 512:768],
                                    osb[:, 512:768])

            # ---- schedule ----
            qk_done = [set(), set()]

            def need_qk(s, h):
                j = h // 2
                if j < CC6 and j not in qk_done[s]:
                    qk_done[s].add(j)
                    emit_qkgen(s, j)

            emit_weight_dmas()
            need_qk(0, 0)
            emit_scores(0, 0)
            emit_v(0, 0)
            need_qk(0, 2)
            emit_scores(0, 1)
            emit_v(0, 1)
            emit_scores(0, 2)
            emit_v(0, 2)
            emit_scores(0, 3)
            emit_x_dmas(1)
            emit_v(0, 3)

            # slice 0 attention; slice 1 qkgen/v interleaved
            e1 = ([(need_qk, 1, 0), (need_qk, 1, 2)]
                  + [(emit_v, 1, mc) for mc in range(NC4)]
                  + [(need_qk, 1, 4), (need_qk, 1, 6),
                     (need_qk, 1, 8), (need_qk, 1, 10)])
            k = 0
            for h in range(H):
                if h + 4 < H:
                    need_qk(0, h + 5)
                    emit_scores(0, h + 4)
                else:
                    emit_scores(1, h + 4 - H)
                emit_pv(0, h)
                if h == 3:
                    emit_projw_dmas()
                tgt = (len(e1) * (h + 1)) // H
                while k < tgt:
                    f, a, b = e1[k]; f(a, b); k += 1

            # slice 1 attention; slice 0 outT+proj interleaved
            GRP_CCS = {2: [0], 5: [1, 2], 8: [3], 11: [4, 5]}
            p0 = ([(emit_outT, 0, n4, list(range(CC6))) for n4 in range(NC4)]
                  + [(emit_proj, 0, n4) for n4 in range(NC4)])
            k = 0
            for h in range(H):
                if h + 4 < H:
                    emit_scores(1, h + 4)
                emit_pv(1, h)
                if h in GRP_CCS and h != 11:
                    for n4 in range(NC4):
                        emit_outT(1, n4, GRP_CCS[h])
                tgt = (len(p0) * (h + 1)) // H
                while k < tgt:
                    u = p0[k]; u[0](*u[1:]); k += 1
            for n4 in range(NC4):
                emit_outT(1, n4, GRP_CCS[11])
                emit_proj(1, n4)

    nc.finalize()
    return nc


def _prep(x, mask, qkv_w, proj_w):
    """Host-side: scale folds, fp8 quantization, pre-transposed layouts."""
    scale = Dh ** -0.5
    wT = np.ascontiguousarray(qkv_w.T).astype(np.float32)   # [C, 3C]
    wT[:, :C] *= scale * QSq
    wT[:, C:2 * C] *= QSk
    wqk = wT[:, :2 * C]                                     # [C, 2C] scaled
    wv = wT[:, 2 * C:]                                      # [C, C] raw

    def plane_pack(w):  # [C, D] -> [CP3, 128, 2*D] (plane-major free dim)
        D = w.shape[1]
        v = w.reshape(CP3, 2, 128, D).transpose(0, 2, 1, 3)
        return np.ascontiguousarray(v.reshape(CP3, 128, 2 * D))

    wqk8 = plane_pack(wqk).astype(NP8)
    wv1 = (wv * VS).astype(np.float32)
    wv8 = wv1.astype(NP8)
    wv8r = (wv1 - wv8.astype(np.float32)).astype(NP8)
    wv8b = (wv * (VS / XRS)).astype(NP8)
    wv8 = plane_pack(wv8.astype(np.float32)).astype(NP8)
    wv8r = plane_pack(wv8r.astype(np.float32)).astype(NP8)
    wv8b = plane_pack(wv8b.astype(np.float32)).astype(NP8)

    wpb = np.ascontiguousarray(proj_w.T).astype(np.float32).astype(NPBF)

    x = x.reshape(B * T, N, C).astype(np.float32)
    x8 = x.astype(NP8)
    x8r = ((x - x8.astype(np.float32)) * XRS).astype(NP8)

    def xt_pack(a):  # [BT, N, C] fp8 -> [BT, 128, CC6*N]
        v = a.reshape(B * T, N, CC6, 128).transpose(0, 3, 2, 1)
        return np.ascontiguousarray(v.reshape(B * T, 128, CC6 * N))

    xt8 = xt_pack(x8)
    xt8r = xt_pack(x8r)

    em = np.exp(mask.reshape(N, N).T.astype(np.float32))     # [m, n]
    emw = np.ascontiguousarray(
        em.reshape(NC4, 128, N).transpose(1, 0, 2).reshape(128, NC4 * N)
    ).astype(NPBF)
    return xt8, xt8r, wqk8, wv8, wv8r, wv8b, wpb, emw


def make_sim_feed(inputs, core=0):
    x = np.asarray(inputs["x"]).astype(np.float32)
    mask = np.asarray(inputs["mask"])
    qkv_w = np.asarray(inputs["qkv_w"]).astype(np.float32)
    proj_w = np.asarray(inputs["proj_w"]).astype(np.float32)
    xt8, xt8r, wqk8, wv8, wv8r, wv8b, wpb, emw = _prep(x, mask, qkv_w, proj_w)
    return {"xt8": xt8[SL * core:SL * (core + 1)],
            "xt8r": xt8r[SL * core:SL * (core + 1)],
            "wqk8": wqk8, "wv8": wv8, "wv8r": wv8r, "wv8b": wv8b,
            "wp": wpb, "emw": emw}


def kernel(x, mask, qkv_w, q_bias, v_bias, proj_w, proj_b,
           _trace=False, _trace_kwargs=None):
    x, mask, qkv_w, proj_w = (np.asarray(a) for a in (x, mask, qkv_w, proj_w))
    q_bias, v_bias, proj_b = (np.asarray(a) for a in (q_bias, v_bias, proj_b))
    assert not np.any(q_bias) and not np.any(v_bias) and not np.any(proj_b), \
        "nonzero biases not supported by this kernel build"
    xt8, xt8r, wqk8, wv8, wv8r, wv8b, wpb, emw = _prep(
        x.astype(np.float32), mask, qkv_w.astype(np.float32),
        proj_w.astype(np.float32))

    if "nc" not in _cache:
        _cache["nc"] = build_nc()
    nc = _cache["nc"]

    in_maps = []
    for c in range(NCORES):
        in_maps.append({
            "xt8": xt8[SL * c:SL * (c + 1)],
            "xt8r": xt8r[SL * c:SL * (c + 1)],
            "wqk8": wqk8, "wv8": wv8, "wv8r": wv8r, "wv8b": wv8b,
            "wp": wpb, "emw": emw,
        })
    res = run_bass_kernel_spmd(
        nc, in_maps, core_ids=list(range(NCORES)),
        trace=_trace, **(_trace_kwargs or {}),
    )
    out = np.concatenate([res.results[c]["y"] for c in range(NCORES)], axis=0)
    out = out.reshape(B, T, N, C)
    if _trace:
        return out, res
    return out


# revision 29
# speedup vs baseline: 1.0100x; 1.0100x over previous
"""Trainium2 Bass kernel for nn_Attention (B=2,T=8,N=512,C=768,H=12).

Data-parallel: 16 (b,t) slices -> 2 per core x 8 cores. Scheme (b2):
  - x uploaded PRE-TRANSPOSED from host in fp8 (xT8) + scaled fp8
    residual (xT8r): no on-device transposes/copies for x.
  - qk-gen: fp8 DoubleRow 1-pass (256-contraction, 0.5 cy/col):
    q psum = x8 @ (Wq*scale*QSq), k psum = x8 @ (Wk*QSk); drained to
    plain BF16 q/k tiles (exact logits up to the fp8 matmul noise).
  - scores: plain bf16 matmuls (64-contraction): S = kb^T qb; costs the
    same PE as the DR alternative but halves the psum drains and error.
  - v-gen: fp8 DR 3-pass (x8@Wv*VS + x8@Wv*VSres + x8r@Wv*VS/XRS) ->
    v*VS bf16; l-column = VS so softmax normalization cancels the scale.
  - pv/outT/proj: bf16.
PE ~81600 cy/slice (vs 104640 baseline). ACT = exp only (+ a few
copies), drains batched wide on DVE, mask-mults split DVE/Pool.
"""
import sys

sys.path.insert(0, "/opt/trn_rl_repo")

import numpy as np
import ml_dtypes
import concourse.bacc as bacc
import concourse.mybir as mybir
import concourse.tile as tile
from concourse.bass import AP, broadcast_tensor_aps
from concourse.bass_utils import run_bass_kernel_spmd
from concourse.masks import make_identity

B, T, N, C = 2, 8, 512, 768
H = 12
Dh = C // H            # 64
SL = 2                 # slices per core
NCORES = 8
NC4 = N // 128         # 4 n-chunks
CC6 = C // 128         # 6 c-chunks
CP3 = CC6 // 2         # 3 c-chunk pairs (DoubleRow planes)
F32 = mybir.dt.float32
BF16 = mybir.dt.bfloat16
FP8 = mybir.dt.float8e4
EXP = mybir.ActivationFunctionType.Exp
DR = mybir.MatmulPerfMode.DoubleRow
NP8 = ml_dtypes.float8_e4m3
NPBF = ml_dtypes.bfloat16

QSq = 512.0            # q psum scale (incl. Dh^-0.5 fold)
QSk = 64.0             # k psum scale
XRS = 8.0              # x residual upscale
VS = 512.0             # v psum scale (cancelled via l-column = VS)
EXPSCALE = 1.0 / (QSq * QSk)

_cache = {}


def build_nc():
    nc = bacc.Bacc()
    xt8 = nc.dram_tensor("xt8", [SL, 128, CC6 * N], FP8, kind="ExternalInput")
    xt8r = nc.dram_tensor("xt8r", [SL, 128, CC6 * N], FP8, kind="ExternalInput")
    wqk8 = nc.dram_tensor("wqk8", [CP3, 128, 2 * 2 * C], FP8, kind="ExternalInput")
    wv8 = nc.dram_tensor("wv8", [CP3, 128, 2 * C], FP8, kind="ExternalInput")
    wv8r = nc.dram_tensor("wv8r", [CP3, 128, 2 * C], FP8, kind="ExternalInput")
    wv8b = nc.dram_tensor("wv8b", [CP3, 128, 2 * C], FP8, kind="ExternalInput")
    wp = nc.dram_tensor("wp", [C, C], BF16, kind="ExternalInput")
    emw = nc.dram_tensor("emw", [128, NC4 * N], BF16, kind="ExternalInput")
    y = nc.dram_tensor("y", [SL, N, C], F32, kind="ExternalOutput")

    with tile.TileContext(nc) as tc:
        with (
            tc.tile_pool(name="wpool", bufs=1) as wpool,
            tc.tile_pool(name="sb", bufs=1) as sb,
            tc.tile_pool(name="ps", bufs=1, space="PSUM") as ps,
        ):
            # ---- persistent weights / inputs ----
            qkw = [wpool.tile([128, 2 * 2 * C], FP8, tag=f"qkw{p}", name=f"qkw{p}")
                   for p in range(CP3)]
            vw = [wpool.tile([128, 2 * C], FP8, tag=f"vw{p}", name=f"vw{p}")
                  for p in range(CP3)]
            vwr = [wpool.tile([128, 2 * C], FP8, tag=f"vwr{p}", name=f"vwr{p}")
                   for p in range(CP3)]
            vwb = [wpool.tile([128, 2 * C], FP8, tag=f"vwb{p}", name=f"vwb{p}")
                   for p in range(CP3)]
            projw = [wpool.tile([128, C], BF16, tag=f"projw{cc}", name=f"projw{cc}")
                     for cc in range(CC6)]
            emwt = wpool.tile([128, NC4 * N], BF16, tag="emw", name="emw")
            xts = [wpool.tile([128, CC6 * N], FP8, tag=f"xt{s}", name=f"xt{s}")
                   for s in range(SL)]
            xtrs = [wpool.tile([128, CC6 * N], FP8, tag=f"xtr{s}", name=f"xtr{s}")
                    for s in range(SL)]

            identf = wpool.tile([128, 128], F32, tag="identf", name="identf")
            make_identity(nc, identf[:])
            ident = wpool.tile([128, 128], BF16, tag="ident", name="ident")
            with nc.allow_low_precision(reason="bf16 identity"):
                nc.vector.tensor_copy(ident[:], identf[:])
            # warm the exp table early so the first scores-exp is fast
            warm = wpool.tile([128, 1], F32, tag="warm", name="warm")
            nc.scalar.activation(warm[:], identf[:, 0:1], EXP)

            def emit_weight_dmas():
                # critical first: qkw + xt0 spread across 4 engine sequencers
                nc.sync.dma_start(qkw[0][:], wqk8[0, :, :])
                nc.scalar.dma_start(qkw[1][:], wqk8[1, :, :])
                nc.gpsimd.dma_start(qkw[2][:], wqk8[2, :, :])
                hw = CC6 * N // 2
                nc.gpsimd.dma_start(xts[0][:, 0:hw], xt8[0, :, 0:hw])
                nc.gpsimd.dma_start(xts[0][:, hw:], xt8[0, :, hw:])
                nc.gpsimd.dma_start(emwt[:], emw[:, :])
                # v weights + x residual next (needed a few us in)
                for p in range(CP3):
                    nc.sync.dma_start(vw[p][:], wv8[p, :, :])
                    nc.sync.dma_start(vwr[p][:], wv8r[p, :, :])
                    nc.sync.dma_start(vwb[p][:], wv8b[p, :, :])
                nc.gpsimd.dma_start(xtrs[0][:, 0:hw], xt8r[0, :, 0:hw])
                nc.gpsimd.dma_start(xtrs[0][:, hw:], xt8r[0, :, hw:])

            def emit_projw_dmas():
                for cc in range(CC6):
                    nc.gpsimd.dma_start(projw[cc][:], wp[128 * cc:128 * (cc + 1), :])

            def emit_x_dmas(s):
                hw = CC6 * N // 2
                nc.sync.dma_start(xts[s][:, 0:hw], xt8[s, :, 0:hw])
                nc.sync.dma_start(xts[s][:, hw:], xt8[s, :, hw:])
                nc.sync.dma_start(xtrs[s][:, 0:hw], xt8r[s, :, 0:hw])
                nc.sync.dma_start(xtrs[s][:, hw:], xt8r[s, :, hw:])

            # ---- per-slice state ----
            cnt = {"msk": 0, "kb": 0}
            qbs = [[None] * CC6 for _ in range(SL)]
            kbs = [[None] * CC6 for _ in range(SL)]
            v3s = [[None] * NC4 for _ in range(SL)]
            outs = [[None] * NC4 for _ in range(SL)]
            outTs = [None] * SL
            ptiles = {}
            pots = [None, None]

            def get(lst, i, mk):
                if lst[i] is None:
                    lst[i] = mk()
                return lst[i]

            def xview(s, resid=False):
                t = (xtrs if resid else xts)[s]
                return t[:].rearrange("p (g n) -> p g n", g=CC6)

            def emit_qkgen(s, j):
                """q,k chunk j (heads 2j,2j+1): 6 DR matmuls + 2 bf16 drains."""
                pq = ps.tile([128, 1024], F32, tag="big", name=f"pqk{s}_{j}", bufs=3)
                xv = xview(s)
                for i in range(CP3):
                    ccp = (j + i) % CP3
                    rhs = xv[:, 2 * ccp:2 * ccp + 2, :]
                    wv_ = qkw[ccp][:].rearrange("p (u d) -> p u d", u=2)
                    nc.tensor.matmul(pq[:, 0:512], wv_[:, :, 128 * j:128 * (j + 1)],
                                     rhs, start=(i == 0), stop=(i == CP3 - 1),
                                     perf_mode=DR)
                    nc.tensor.matmul(pq[:, 512:1024],
                                     wv_[:, :, C + 128 * j:C + 128 * (j + 1)],
                                     rhs, start=(i == 0), stop=(i == CP3 - 1),
                                     perf_mode=DR)
                qb = get(qbs[s], j, lambda: sb.tile(
                    [128, N], BF16, tag="qb", name=f"qb_{s}_{j}", bufs=7))
                kb = get(kbs[s], j, lambda: sb.tile(
                    [128, N], BF16, tag="kb", name=f"kb_{s}_{j}", bufs=7))
                with nc.allow_low_precision(reason="bf16 q/k"):
                    nc.vector.tensor_copy(qb[:], pq[:, 0:512])
                    nc.vector.tensor_copy(kb[:], pq[:, 512:1024])

            def emit_v(s, mc):
                """v*VS for token chunk mc: 18 DR matmuls (3 passes) + 1 drain."""
                pv = ps.tile([128, 1024], F32, tag="big", name=f"pv{s}_{mc}", bufs=3)
                va, vb = pv[:, 0:512], pv[:, 512:768]
                xv, xvr = xview(s), xview(s, resid=True)
                steps = []
                for w_ in (vw, vwr):
                    for i in range(CP3):
                        steps.append((xv, w_[(mc + i) % CP3], (mc + i) % CP3))
                for i in range(CP3):
                    steps.append((xvr, vwb[(mc + i) % CP3], (mc + i) % CP3))
                for i, (xsrc, w, ccp) in enumerate(steps):
                    lhsT = xsrc[:, 2 * ccp:2 * ccp + 2, 128 * mc:128 * (mc + 1)]
                    wv_ = w[:].rearrange("p (u d) -> p u d", u=2)
                    nc.tensor.matmul(va, lhsT, wv_[:, :, 0:512],
                                     start=(i == 0), stop=(i == len(steps) - 1),
                                     perf_mode=DR)
                    nc.tensor.matmul(vb, lhsT, wv_[:, :, 512:768],
                                     start=(i == 0), stop=(i == len(steps) - 1),
                                     perf_mode=DR)
                v3 = get(v3s[s], mc, lambda: sb.tile(
                    [128, H * (Dh + 1)], BF16, tag="v3", name=f"v3_{s}_{mc}", bufs=8))
                v3r = v3[:].rearrange("p (h e) -> p h e", e=Dh + 1)
                with nc.allow_low_precision(reason="bf16 v"):
                    nc.vector.tensor_copy(
                        v3r[:, :, 0:Dh],
                        pv[:, 0:C].rearrange("p (h e) -> p h e", e=Dh))
                nc.gpsimd.memset(v3r[:, :, Dh:Dh + 1], VS)

            def emit_scores(s, h):
                """S^T for head h (bf16) -> exp -> P[128, 2048] (col 512*mc+n)."""
                j = h // 2
                hb = 64 * (h % 2)
                qb = qbs[s][j][hb:hb + 64, :]
                kb = kbs[s][j]
                ptile = sb.tile([128, NC4 * N], BF16, tag="ptile",
                                name=f"pt{s}_{h}", bufs=7)
                for half in range(2):
                    pst = ps.tile([128, 1024], F32, tag="big",
                                  name=f"ps{s}_{h}_{half}", bufs=3)
                    for m2 in range(2):
                        mc = 2 * half + m2
                        nc.tensor.matmul(pst[:, 512 * m2:512 * (m2 + 1)],
                                         kb[hb:hb + 64, 128 * mc:128 * (mc + 1)],
                                         qb, start=True, stop=True)
                    nc.scalar.activation(ptile[:, 1024 * half:1024 * (half + 1)],
                                         pst[:], EXP, scale=EXPSCALE)
                with nc.allow_low_precision(reason="bf16 P"):
                    nc.gpsimd.tensor_mul(ptile[:], ptile[:], emwt[:])
                ptiles[(s, h)] = ptile

            def emit_pv(s, h):
                v3 = v3s[s]
                g = h // 3
                hg = h % 3
                ptile = ptiles.pop((s, h))
                if hg == 0:
                    for p in range(2):
                        pots[p] = ps.tile([128, 2 * 3 * (Dh + 1)], F32, tag="pot",
                                          name=f"pot{s}_{g}_{p}", bufs=2)
                for n4 in range(NC4):
                    pot = pots[n4 // 2][:, 195 * (n4 % 2):195 * (n4 % 2) + 195]
                    for mc in range(NC4):
                        nc.tensor.matmul(pot[:, 65 * hg:65 * (hg + 1)],
                                         ptile[:, 512 * mc + 128 * n4:
                                               512 * mc + 128 * (n4 + 1)],
                                         v3[mc][:, 65 * h:65 * (h + 1)],
                                         start=(mc == 0), stop=(mc == NC4 - 1),
                                         skip_group_check=True)
                if hg == 2:
                    for n4 in range(NC4):
                        pot = pots[n4 // 2][:, 195 * (n4 % 2):195 * (n4 % 2) + 195]
                        potv = pot.rearrange("p (h e) -> p h e", e=Dh + 1)
                        lr = sb.tile([128, 3], F32, tag="lr",
                                     name=f"lr{s}_{g}_{n4}", bufs=4)
                        nc.vector.reciprocal(
                            lr[:].rearrange("p (h e) -> p h e", e=1),
                            potv[:, :, Dh:Dh + 1])
                        out = get(outs[s], n4, lambda n4=n4: sb.tile(
                            [128, C], BF16, tag="out", name=f"out_{s}_{n4}", bufs=8))
                        b0, b1 = broadcast_tensor_aps(
                            potv[:, :, 0:Dh],
                            lr[:].rearrange("p (h e) -> p h e", e=1))
                        with nc.allow_low_precision(reason="bf16 out"):
                            nc.vector.tensor_mul(
                                out[:, 192 * g:192 * (g + 1)].rearrange(
                                    "p (h e) -> p h e", e=Dh), b0, b1)

            def emit_outT(s, n4, ccs):
                out = outs[s][n4]
                outT = get(outTs, s, lambda: sb.tile(
                    [128, CC6 * N], BF16, tag="outT", name=f"outT_{s}", bufs=2))
                pt = ps.tile([128, 1024], BF16, tag="big",
                             name=f"ot{s}_{n4}_{min(ccs)}", bufs=3)
                for i, cc in enumerate(ccs):
                    nc.tensor.transpose(pt[:, 128 * i:128 * (i + 1)],
                                        out[:, 128 * cc:128 * (cc + 1)], ident[:])
                oTv = outT[:].rearrange("p (cc n) -> p cc n", cc=CC6)
                dst = oTv[:, min(ccs):min(ccs) + len(ccs),
                          128 * n4:128 * (n4 + 1)]
                src = pt[:, 0:128 * len(ccs)].rearrange(
                    "p (cc n) -> p cc n", n=128)
                with nc.allow_low_precision(reason="bf16 outT"):
                    nc.vector.tensor_copy(dst, src)

            def emit_proj(s, n4):
                outT = outTs[s]
                oTv = outT[:].rearrange("p (cc n) -> p cc n", cc=CC6)
                osb = sb.tile([128, C], F32, tag="osb", name=f"osb{s}_{n4}", bufs=3)
                if s == 1 and n4 == NC4 - 1:
                    # final unit: 3 narrow psum groups so the drain pipelines
                    bounds = [(0, 320), (320, 640), (640, 768)]
                    for third, (c0, c1) in enumerate(bounds):
                        w = c1 - c0
                        pr = ps.tile([128, 1024], F32, tag="big",
                                     name=f"pr{s}_{n4}_{third}", bufs=3)
                        for cc in range(CC6):
                            lhsT = oTv[:, cc, 128 * n4:128 * (n4 + 1)]
                            nc.tensor.matmul(pr[:, 0:w], lhsT, projw[cc][:, c0:c1],
                                             start=(cc == 0), stop=(cc == CC6 - 1))
                        ceng = (nc.vector.tensor_copy, nc.scalar.copy,
                                nc.vector.tensor_copy)[third]
                        ceng(osb[:, c0:c1], pr[:, 0:w])
                        deng = (nc.sync, nc.gpsimd, nc.scalar)[third]
                        deng.dma_start(y[s, 128 * n4:128 * (n4 + 1), c0:c1],
                                       osb[:, c0:c1])
                    return
                pr = ps.tile([128, 1024], F32, tag="big",
                             name=f"pr{s}_{n4}", bufs=3)
                pra, prb = pr[:, 0:512], pr[:, 512:768]
                for cc in range(CC6):
                    lhsT = oTv[:, cc, 128 * n4:128 * (n4 + 1)]
                    nc.tensor.matmul(pra, lhsT, projw[cc][:, 0:512],
                                     start=(cc == 0), stop=(cc == CC6 - 1))
                    nc.tensor.matmul(prb, lhsT, projw[cc][:, 512:768],
                                     start=(cc == 0), stop=(cc == CC6 - 1))
                nc.vector.tensor_copy(osb[:, 0:512], pra)
                nc.sync.dma_start(y[s, 128 * n4:128 * (n4 + 1), 0:512],
                                  osb[:, 0:512])
                nc.vector.tensor_copy(osb[:, 512:768], prb)
                nc.gpsimd.dma_start(y[s, 128 * n4:128 * (n4 + 1), 512:768],
                                    osb[:, 512:768])

            # ---- schedule ----
            qk_done = [set(), set()]

            def need_qk(s, h):
                j = h // 2
                if j < CC6 and j not in qk_done[s]:
                    qk_done[s].add(j)
                    emit_qkgen(s, j)

            emit_weight_dmas()
            need_qk(0, 0)
            emit_scores(0, 0)
            emit_v(0, 0)
            need_qk(0, 2)
            emit_scores(0, 1)
            emit_v(0, 1)
            emit_scores(0, 2)
            emit_v(0, 2)
            emit_scores(0, 3)
            emit_x_dmas(1)
            emit_v(0, 3)

            # slice 0 attention; slice 1 qkgen/v interleaved
            e1 = ([(need_qk, 1, 0), (need_qk, 1, 2)]
                  + [(emit_v, 1, mc) for mc in range(NC4)]
                  + [(need_qk, 1, 4), (need_qk, 1, 6),
                     (need_qk, 1, 8), (need_qk, 1, 10)])
            k = 0
            for h in range(H):
                if h + 4 < H:
                    need_qk(0, h + 5)
                    emit_scores(0, h + 4)
                else:
                    emit_scores(1, h + 4 - H)
                emit_pv(0, h)
                if h == 3:
                    emit_projw_dmas()
                tgt = (len(e1) * (h + 1)) // H
                while k < tgt:
                    f, a, b = e1[k]; f(a, b); k += 1

            # slice 1 attention; slice 0 outT+proj interleaved
            GRP_CCS = {2: [0], 5: [1, 2], 8: [3], 11: [4, 5]}
            p0 = ([(emit_outT, 0, n4, list(range(CC6))) for n4 in range(NC4)]
                  + [(emit_proj, 0, n4) for n4 in range(NC4)])
            k = 0
            for h in range(H):
                if h + 4 < H:
                    emit_scores(1, h + 4)
                emit_pv(1, h)
                if h in GRP_CCS and h != 11:
                    for n4 in range(NC4):
                        emit_outT(1, n4, GRP_CCS[h])
                tgt = (len(p0) * (h + 1)) // H
                while k < tgt:
                    u = p0[k]; u[0](*u[1:]); k += 1
            for n4 in range(NC4):
                emit_outT(1, n4, GRP_CCS[11])
                emit_proj(1, n4)

    nc.finalize()
    return nc


def _prep(x, mask, qkv_w, proj_w):
    """Host-side: scale folds, fp8 quantization, pre-transposed layouts."""
    scale = Dh ** -0.5
    wT = np.ascontiguousarray(qkv_w.T).astype(np.float32)   # [C, 3C]
    wT[:, :C] *= scale * QSq
    wT[:, C:2 * C] *= QSk
    wqk = wT[:, :2 * C]                                     # [C, 2C] scaled
    wv = wT[:, 2 * C:]                                      # [C, C] raw

    def plane_pack(w):  # [C, D] -> [CP3, 128, 2*D] (plane-major free dim)
        D = w.shape[1]
        v = w.reshape(CP3, 2, 128, D).transpose(0, 2, 1, 3)
        return np.ascontiguousarray(v.reshape(CP3, 128, 2 * D))

    wqk8 = plane_pack(wqk).astype(NP8)
    wv1 = (wv * VS).astype(np.float32)
    wv8 = wv1.astype(NP8)
    wv8r = (wv1 - wv8.astype(np.float32)).astype(NP8)
    wv8b = (wv * (VS / XRS)).astype(NP8)
    wv8 = plane_pack(wv8.astype(np.float32)).astype(NP8)
    wv8r = plane_pack(wv8r.astype(np.float32)).astype(NP8)
    wv8b = plane_pack(wv8b.astype(np.float32)).astype(NP8)

    wpb = np.ascontiguousarray(proj_w.T).astype(np.float32).astype(NPBF)

    x = x.reshape(B * T, N, C).astype(np.float32)
    x8 = x.astype(NP8)
    x8r = ((x - x8.astype(np.float32)) * XRS).astype(NP8)

    def xt_pack(a):  # [BT, N, C] fp8 -> [BT, 128, CC6*N]
        v = a.reshape(B * T, N, CC6, 128).transpose(0, 3, 2, 1)
        return np.ascontiguousarray(v.reshape(B * T, 128, CC6 * N))

    xt8 = xt_pack(x8)
    xt8r = xt_pack(x8r)

    em = np.exp(mask.reshape(N, N).T.astype(np.float32))     # [m, n]
    emw = np.ascontiguousarray(
        em.reshape(NC4, 128, N).transpose(1, 0, 2).reshape(128, NC4 * N)
    ).astype(NPBF)
    return xt8, xt8r, wqk8, wv8, wv8r, wv8b, wpb, emw


def make_sim_feed(inputs, core=0):
    x = np.asarray(inputs["x"]).astype(np.float32)
    mask = np.asarray(inputs["mask"])
    qkv_w = np.asarray(inputs["qkv_w"]).astype(np.float32)
    proj_w = np.asarray(inputs["proj_w"]).astype(np.float32)
    xt8, xt8r, wqk8, wv8, wv8r, wv8b, wpb, emw = _prep(x, mask, qkv_w, proj_w)
    return {"xt8": xt8[SL * core:SL * (core + 1)],
            "xt8r": xt8r[SL * core:SL * (core + 1)],
            "wqk8": wqk8, "wv8": wv8, "wv8r": wv8r, "wv8b": wv8b,
            "wp": wpb, "emw": emw}


def kernel(x, mask, qkv_w, q_bias, v_bias, proj_w, proj_b,
           _trace=False, _trace_kwargs=None):
    x, mask, qkv_w, proj_w = (np.asarray(a) for a in (x, mask, qkv_w, proj_w))
    q_bias, v_bias, proj_b = (np.asarray(a) for a in (q_bias, v_bias, proj_b))
    assert not np.any(q_bias) and not np.any(v_bias) and not np.any(proj_b), \
        "nonzero biases not supported by this kernel build"
    xt8, xt8r, wqk8, wv8, wv8r, wv8b, wpb, emw = _prep(
        x.astype(np.float32), mask, qkv_w.astype(np.float32),
        proj_w.astype(np.float32))

    if "nc" not in _cache:
        _cache["nc"] = build_nc()
    nc = _cache["nc"]

    in_maps = []
    for c in range(NCORES):
        in_maps.append({
            "xt8": xt8[SL * c:SL * (c + 1)],
            "xt8r": xt8r[SL * c:SL * (c + 1)],
            "wqk8": wqk8, "wv8": wv8, "wv8r": wv8r, "wv8b": wv8b,
            "wp": wpb, "emw": emw,
        })
    res = run_bass_kernel_spmd(
        nc, in_maps, core_ids=list(range(NCORES)),
        trace=_trace, **(_trace_kwargs or {}),
    )
    out = np.concatenate([res.results[c]["y"] for c in range(NCORES)], axis=0)
    out = out.reshape(B, T, N, C)
    if _trace:
        return out, res
    return out
